# revision 68
# baseline (speedup 1.0000x reference)
"""GroupedQueryAttention Trainium2 kernel (8 NeuronCores) — v2.

Problem: B=4, S=N=2048, d_model=2048, G=16 heads, d_head=128,
RoPE (rotary_dim=512) applied to query only, key mask,
out = (softmax(mask(QK^T/sqrt(dh))) @ V) @ Wo^T.

Sharding: mesh = 4 batches x 2 head-halves. core_id = b*2 + h.
Each core: projections for its batch/head-half, attention for its 8 heads,
pair AllGather of context^T (chunked by query block, overlapped with
compute), O-projection of its 1024 output columns.

Changes vs the fp32r baseline (timeline-sim 1382us -> 768us):
  - fp16 operands everywhere (PSUM accumulation stays fp32). Halves DMA
    bytes and SBUF footprint, enables the 2x DVE mode. rel-err budget is
    2e-2; fp16 lands ~9e-4 on HW.
  - The key mask is folded into the projections: masked keys' k_sb columns
    and v_sb rows are zeroed at PSUM->SBUF copy time. Masked scores are
    then exactly 0 and exp gives exactly 1, so the softmax denominator is
    (sum of all e) - (masked-key count), with the count fed per-core as a
    constant. exp needs no bias, so one activation instruction covers two
    key tiles (a 2-bank-wide PSUM read), amortizing Act fixed costs.
  - Softmax denominator: e tiles accumulate on DVE (2x fp16 adds), a gpsimd
    partition_all_reduce produces the broadcast key-sum, DVE subtracts the
    masked count and takes the reciprocal. Removes the ones-vector matmul
    chain (~109us of PE) and frees two PSUM banks.
  - Attention score matmuls are software-pipelined one key-pair ahead of
    their exp so the in-order PE queue never blocks on the Act engine.
  - Query chunks are processed last-first: the final Q-projection chunk's
    rope'd tiles stay in SBUF and attention starts on them immediately.
  - The context AllGather is chunked per 512-query block and overlaps the
    following attention chunks. With the denominator off the PE, attention
    is Act-engine-paced, so ALL O-projection work trails the attention
    phase: keeping O matmuls out of the in-order PE queue lets each
    chunk's stores (which gate its data-dependent gather) complete at
    act speed, and the collective chain finishes just as the trailing
    O-projection reaches its chunk. Gathered-context loads are pinned with
    tile_wait_until near each collective's completion so the scheduler
    cannot hoist them into a DMA queue where they would head-of-line block
    (the single biggest baseline stall was exactly that: a ~270us dead
    window behind one monolithic gather).
  - Weights live in a 2-slot SBUF ring (wk->wv->wq->wo): the next phase's
    weight DMA streams on the Act-engine HWDGE queue while the current
    phase computes on x-tiles streamed via the SP queue. First tiles of
    each phase are prefetched one phase early through a shared x-tile ring.
  - K/V/Q projections share one 8-bank PSUM pool; the Q phase rotates the
    head->bank mapping per chunk and drains copies before rope so the next
    chunk's accumulation starts without waiting on the DVE tail.
"""
import sys
import numpy as np

sys.path.insert(0, "/opt/trn_rl_repo")

import concourse.bass as bass
import concourse.tile as tile
from concourse import bacc, bass_isa, mybir
from concourse.bass_utils import run_bass_kernel_spmd

FP32 = mybir.dt.float32
FP16 = mybir.dt.float16

B = 4
S = 2048          # queries per batch
N = 2048          # keys per batch
D = 2048          # d_model
G = 16            # heads
DH = 128          # head dim
RD = 512          # rotary dim
TP = 2            # head-half split
CL = D // TP      # local channels (1024)
GL = G // TP      # local heads (8)
OC = D // TP      # output cols per core (1024)
SCALE = 1.0 / float(np.sqrt(DH))

KT = D // 128     # contraction k-tiles (16)
SC = S // 512     # query chunks (4)
NT = N // 128     # key tiles (16)
ST = S // 128     # query 128-tiles (16)
CT = D // 128     # context c-tiles (16)

LAST_RESULT = None


def _build_program():
    nc = bacc.Bacc("TRN2", target_bir_lowering=False, debug=False, num_devices=8)

    # ---- external I/O (per-core contents differ; same shapes) ----
    xq = nc.dram_tensor("xq", [D, S], FP16, kind="ExternalInput").ap()    # query^T
    xk = nc.dram_tensor("xk", [D, N], FP16, kind="ExternalInput").ap()    # key^T
    xv = nc.dram_tensor("xv", [D, N], FP16, kind="ExternalInput").ap()    # value^T
    wq = nc.dram_tensor("wq", [D, CL], FP16, kind="ExternalInput").ap()   # Wq[hs,:]^T
    wk = nc.dram_tensor("wk", [D, CL], FP16, kind="ExternalInput").ap()
    wv = nc.dram_tensor("wv", [D, CL], FP16, kind="ExternalInput").ap()
    wo = nc.dram_tensor("wo", [D, OC], FP16, kind="ExternalInput").ap()   # Wo^T[:, ocs]
    cosT = nc.dram_tensor("cosT", [RD, S], FP16, kind="ExternalInput").ap()
    sinT = nc.dram_tensor("sinT", [RD, S], FP16, kind="ExternalInput").ap()  # signed
    mvec = nc.dram_tensor("mvec", [128, NT], FP32, kind="ExternalInput").ap()
    cvec = nc.dram_tensor("cvec", [128, 1], FP32, kind="ExternalInput").ap()
    mkey = nc.dram_tensor("mkey", [128, N], FP16, kind="ExternalInput").ap()
    out = nc.dram_tensor("out", [S, OC], FP16, kind="ExternalOutput").ap()

    # ---- DRAM scratch ----
    qT_d = nc.dram_tensor("qT_d", [CL, S], FP16).ap()  # rope'd Q^T spill
    ct_loc = [nc.dram_tensor(f"ct_loc{sc}", [CL, 512], FP16).ap() for sc in range(SC)]
    ct_gth = [nc.dram_tensor(f"ct_gth{sc}", [D, 512], FP16).ap()
              for sc in range(SC)]

    xq_r = xq.rearrange("(kt p) s -> p kt s", p=128)
    xk_r = xk.rearrange("(kt p) s -> p kt s", p=128)
    xv_r = xv.rearrange("(kt p) s -> p kt s", p=128)
    wk_r = wk.rearrange("(kt p) c -> p kt c", p=128)
    wv_r = wv.rearrange("(kt p) c -> p kt c", p=128)
    wq_r = wq.rearrange("(kt p) c -> p kt c", p=128)
    wo_r = wo.rearrange("(ct p) c -> p ct c", p=128)
    REPL = [[0, 1], [2, 3], [4, 5], [6, 7]]

    with tile.TileContext(nc) as tc:
        consts = tc.alloc_tile_pool(name="consts", bufs=1)
        mv_t = consts.tile([128, NT], FP32)
        cv_t = consts.tile([128, 1], FP32)

        # resident tensors + the 2-slot weight ring
        kres = tc.alloc_tile_pool(name="kres", bufs=1)
        k_sb = kres.tile([128, GL, NT, 128], FP16)
        vres = tc.alloc_tile_pool(name="vres", bufs=1)
        v_sb = vres.tile([128, NT, CL], FP16)
        wring = tc.alloc_tile_pool(name="wring", bufs=2)
        qout = tc.alloc_tile_pool(name="qout", bufs=8)
        # x-tile halves shared by the K/V/Q phases ([128, 8, 512] each)
        xshare = tc.alloc_tile_pool(name="xshare", bufs=4)

        def x_halves(src_r, c, fine_first=False):
            ha = xshare.tile([128, 8, 512], FP16, name="xh", tag="x")
            if fine_first:
                nc.sync.dma_start(out=ha[:, 0:1, :],
                                  in_=src_r[:, 0:1, c * 512:(c + 1) * 512])
                nc.sync.dma_start(out=ha[:, 1:2, :],
                                  in_=src_r[:, 1:2, c * 512:(c + 1) * 512])
                nc.sync.dma_start(out=ha[:, 2:8, :],
                                  in_=src_r[:, 2:8, c * 512:(c + 1) * 512])
            else:
                nc.sync.dma_start(out=ha, in_=src_r[:, 0:8, c * 512:(c + 1) * 512])
            hb = xshare.tile([128, 8, 512], FP16, name="xh", tag="x")
            nc.sync.dma_start(out=hb, in_=src_r[:, 8:16, c * 512:(c + 1) * 512])
            return ha, hb

        def load_w(dst, src_r, fine_first=False):
            chunks = [(0, 1), (1, 1), (2, 2), (4, 4), (8, 4), (12, 4)] if fine_first \
                else [(0, 4), (4, 4), (8, 4), (12, 4)]
            for lo, n in chunks:
                nc.scalar.dma_start(
                    out=dst[:, lo:lo + n, :], in_=src_r[:, lo:lo + n, :]
                )

        # ---------- Phase K: K-projection -> k_sb ----------
        wk_t = wring.tile([128, KT, CL], FP16, name="wk_t", tag="w")
        load_w(wk_t, wk_r, fine_first=True)
        nc.scalar.dma_start(out=mv_t, in_=mvec)
        nc.scalar.dma_start(out=cv_t, in_=cvec)
        mkpool = tc.alloc_tile_pool(name="mkpool", bufs=1)
        mkb_t = mkpool.tile([128, N], FP16)
        nc.scalar.dma_start(out=mkb_t, in_=mkey)
        wv_t = wring.tile([128, KT, CL], FP16, name="wv_t", tag="w")
        load_w(wv_t, wv_r)   # prefetch into slot 2 during phase K

        pps8 = tc.alloc_tile_pool(name="pps8", bufs=1, space="PSUM")
        xv_pre = None
        for nch in range(N // 512):
            ha, hb = x_halves(xk_r, nch, fine_first=(nch == 0))
            psums = []
            for g in range(GL):
                p = pps8.tile([128, 512], FP32, name=f"kp{g}", tag=f"p{g}")
                psums.append(p)
            for kt in range(KT):
                for g in range(GL):
                    nc.tensor.matmul(
                        out=psums[g],
                        lhsT=wk_t[:, kt, g * 128:(g + 1) * 128],
                        rhs=(ha if kt < 8 else hb)[:, kt % 8, :],
                        start=(kt == 0),
                        stop=(kt == KT - 1),
                    )
            for g in range(GL):
                # masked keys' k columns are zeroed: their scores become
                # exactly 0 and exp gives exactly 1, corrected later by
                # subtracting the masked-key count from the denominator
                nc.vector.tensor_mul(
                    out=k_sb[:, g, nch * 4:(nch + 1) * 4, :],
                    in0=psums[g].rearrange("p (a b) -> p a b", b=128),
                    in1=mkb_t.rearrange("p (a b) -> p a b", b=128)[
                        :, nch * 4:(nch + 1) * 4, :
                    ],
                )
            if nch == 2:
                xv_pre = x_halves(xv_r, 0)   # prefetch phase V's first chunk
        mkpool.release()

        # ---------- Phase V: V-projection -> v_sb (mask-scaled) ----------
        ropool = tc.alloc_tile_pool(name="ropool", bufs=1)
        cosT_r = cosT.rearrange("(gt p) s -> p gt s", p=128)
        sinT_r = sinT.rearrange("(gt p) s -> p gt s", p=128)
        wq_t = wring.tile([128, KT, CL], FP16, name="wq_t", tag="w")
        load_w(wq_t, wq_r)   # prefetch (waits on wk slot free)

        xq_pre = ro_pre = None
        for nt4 in range(N // 512):
            ha, hb = xv_pre if nt4 == 0 else x_halves(xv_r, nt4)
            for j in range(4):
                nt = nt4 * 4 + j
                psums = []
                for cc in range(2):
                    p = pps8.tile([128, 512], FP32, name=f"vp{cc}",
                                  tag=f"p{(j * 2 + cc) % 8}")
                    psums.append(p)
                for kt in range(KT):
                    for cc in range(2):
                        nc.tensor.matmul(
                            out=psums[cc],
                            lhsT=(ha if kt < 8 else hb)[
                                :, kt % 8, j * 128:(j + 1) * 128
                            ],
                            rhs=wv_t[:, kt, cc * 512:(cc + 1) * 512],
                            start=(kt == 0),
                            stop=(kt == KT - 1),
                        )
                # masked copy: zero rows of masked keys
                for cc in range(2):
                    nc.vector.tensor_scalar(
                        out=v_sb[:, nt, cc * 512:(cc + 1) * 512],
                        in0=psums[cc],
                        scalar1=mv_t[:, nt:nt + 1],
                        scalar2=None,
                        op0=mybir.AluOpType.mult,
                    )
            if nt4 == 2:
                # prefetch phase Q's first chunk + rope slices
                xq_pre = x_halves(xq_r, 0)
                cos0 = ropool.tile([128, 4, 512], FP16, name="cos_t", tag="cos")
                sin0 = ropool.tile([128, 4, 512], FP16, name="sin_t", tag="sin")
                nc.scalar.dma_start(out=cos0, in_=cosT_r[:, :, 0:512])
                nc.scalar.dma_start(out=sin0, in_=sinT_r[:, :, 0:512])
                ro_pre = (cos0, sin0)

        # ---------- Phase Q: Q-projection + RoPE -> qT_d ----------
        rsc = tc.alloc_tile_pool(name="ropescratch", bufs=2)
        qo_last = {}
        for sc in range(SC):
            ssl = slice(sc * 512, (sc + 1) * 512)
            ha, hb = xq_pre if sc == 0 else x_halves(xq_r, sc)
            if sc == 0:
                cos_t, sin_t = ro_pre
            else:
                cos_t = ropool.tile([128, 4, 512], FP16, name="cos_t", tag="cos")
                sin_t = ropool.tile([128, 4, 512], FP16, name="sin_t", tag="sin")
                nc.scalar.dma_start(out=cos_t, in_=cosT_r[:, :, ssl])
                nc.scalar.dma_start(out=sin_t, in_=sinT_r[:, :, ssl])
            rot = 4 * (sc % 2)
            psums = []
            for g in range(GL):
                p = pps8.tile([128, 512], FP32, name=f"qp{g}",
                              tag=f"p{(g + rot) % 8}")
                psums.append(p)
            for kt in range(KT):
                for g in range(GL):
                    nc.tensor.matmul(
                        out=psums[g],
                        lhsT=wq_t[:, kt, g * 128:(g + 1) * 128],
                        rhs=(ha if kt < 8 else hb)[:, kt % 8, :],
                        start=(kt == 0),
                        stop=(kt == KT - 1),
                    )
            for g in (4, 5, 6, 7, 0, 1, 2, 3):
                qo = qout.tile([128, 512], FP16, name="qo", tag="qo")
                if g < 4:
                    sA = rsc.tile([128, 512], FP16, name="ropeA", tag="rA")
                    sB = rsc.tile([128, 512], FP16, name="ropeB", tag="rB")
                    nc.vector.tensor_mul(out=sA, in0=psums[g], in1=cos_t[:, g, :])
                    nc.vector.tensor_mul(out=sB, in0=psums[g ^ 2], in1=sin_t[:, g, :])
                    nc.vector.tensor_add(out=qo, in0=sA, in1=sB)
                else:
                    nc.vector.tensor_copy(out=qo, in_=psums[g])
                if sc == SC - 1:
                    # last chunk's q stays in SBUF; attention starts with it
                    qo_last[g] = qo
                else:
                    nc.sync.dma_start(
                        out=qT_d[g * 128:(g + 1) * 128, ssl], in_=qo
                    )
        pps8.release()
        rsc.release()
        ropool.release()
        xshare.release()

        # ---------- Phase A: attention, chunked by query block ----------
        wo_t = wring.tile([128, CT, OC], FP16, name="wo_t", tag="w")
        load_w(wo_t, wo_r)   # streams in while attention runs

        qld = tc.alloc_tile_pool(name="qld", bufs=4)
        wps = tc.alloc_tile_pool(name="wps", bufs=3, space="PSUM")
        ups = tc.alloc_tile_pool(name="ups", bufs=2, space="PSUM")
        epool = tc.alloc_tile_pool(name="epool", bufs=3)
        awpool = tc.alloc_tile_pool(name="awpool", bufs=2)
        rpool = tc.alloc_tile_pool(name="rpool", bufs=2)
        dpool = tc.alloc_tile_pool(name="dpool", bufs=2)
        cpool = tc.alloc_tile_pool(name="cpool", bufs=3)
        ctpool = tc.alloc_tile_pool(name="ctpool", bufs=6)
        oopool = tc.alloc_tile_pool(name="oout", bufs=2)

        # scheduler pins (ms) for the gathered-context loads: roughly when
        # each chunk's collective completes in the timeline sim
        # (dram tensor, quarter-index within it, global ct tiles, pin ms)
        # pins sit at/just-before each gather's completion in the timeline
        # sim; a pinned load at the queue tail waits on its semaphore with
        # nothing behind it to block.
        GPIN = {3: 0.49, 2: 0.545, 1: 0.61, 0: 0.675}
        LOADPLAN = {
            sc: [(ct_gth[sc], q, [4 * q + i for i in range(4)], GPIN[sc])
                 for q in range(4)]
            for sc in range(SC)
        }

        o_quarters = {}

        def load_o_quarters(sc):
            if sc not in o_quarters:
                cmap = []
                for src_t, qi, globs, pin in LOADPLAN[sc]:
                    c_sb = ctpool.tile([128, 4, 512], FP16, name="c_sb", tag="csb")
                    src_r2 = src_t.rearrange("(ct p) t -> p ct t", p=128)
                    with tc.tile_wait_until(pin):
                        nc.scalar.dma_start(
                            out=c_sb, in_=src_r2[:, qi * 4:(qi + 1) * 4, :]
                        )
                    for j, glob in enumerate(globs):
                        cmap.append((c_sb, j, glob))
                o_quarters[sc] = cmap
            return o_quarters[sc]

        def o_seg(o2, sc, tt, seg, nseg):
            cmap = o_quarters[sc]
            per = CT // nseg
            tsl = slice(tt * 128, (tt + 1) * 128)
            for idx in range(seg * per, (seg + 1) * per):
                tile_, j, glob = cmap[idx]
                for cc in range(2):
                    nc.tensor.matmul(
                        out=o2[:, cc, :],
                        lhsT=tile_[:, j, tsl],
                        rhs=wo_t[:, glob, cc * 512:(cc + 1) * 512],
                        start=(idx == 0),
                        stop=(idx == CT - 1),
                    )

        def o_finish(o2, sc, tt):
            o_sb = oopool.tile([128, 2, 512], FP16, name="o_sb", tag="ob")
            nc.vector.tensor_copy(out=o_sb, in_=o2)
            st = sc * 4 + tt
            nc.sync.dma_start(
                out=out[st * 128:(st + 1) * 128, :],
                in_=o_sb.rearrange("p a b -> p (a b)"),
            )

        def emit_o_tt(sc, tt):
            load_o_quarters(sc)
            o2 = wps.tile([128, 2, 512], FP32, name="o2", tag="mm2")
            o_seg(o2, sc, tt, 0, 1)
            o_finish(o2, sc, tt)

        # process query chunks last-first: sc3's q is still in SBUF
        sc_order = list(range(SC - 1, -1, -1))
        def preload_q(sc):
            q_ts = {}
            for g in range(GL):
                q_t = qld.tile([128, 512], FP16, name="q_t", tag="q")
                nc.scalar.dma_start(
                    out=q_t,
                    in_=qT_d[g * 128:(g + 1) * 128, sc * 512:(sc + 1) * 512],
                )
                q_ts[g] = q_t
            return q_ts

        first_s2 = None
        for i, sc in enumerate(sc_order):
            if sc == SC - 1:
                q_ts = qo_last
            else:
                q_ts = preload_q(sc)
            for g in range(GL):
                gsl = slice(g * 128, (g + 1) * 128)
                q_t = q_ts[g]
                u_ps = ups.tile([128, 512], FP32, name="u_ps", tag="u")
                e_aw = awpool.tile([128, 2, 512], FP16, name="e_aw", tag="ea")
                # software-pipelined: scores for pair jp+1 are emitted before
                # the exp-dependent consumers of pair jp (and the next head's
                # first pair before this head's last PV), so neither the PE
                # nor the Act engine head-of-line blocks on the other.
                s2s = [None] * (NT // 2)

                def emit_scores(jp, g=g, qsrc=None):
                    qt = q_ts[g] if qsrc is None else qsrc[g]
                    s2 = wps.tile([128, 2, 512], FP32, name="s2", tag="mm2")
                    nc.tensor.matmul(
                        out=s2[:, 0, :], lhsT=k_sb[:, g, 2 * jp, :],
                        rhs=qt, start=True, stop=True,
                    )
                    nc.tensor.matmul(
                        out=s2[:, 1, :], lhsT=k_sb[:, g, 2 * jp + 1, :],
                        rhs=qt, start=True, stop=True,
                    )
                    return s2

                s2s[0] = first_s2 if first_s2 is not None else emit_scores(0)
                first_s2 = None
                for jp in range(NT // 2):
                    n0, n1 = 2 * jp, 2 * jp + 1
                    e2 = epool.tile([128, 2, 512], FP16, name="e2", tag="e2")
                    nc.scalar.activation(
                        out=e2, in_=s2s[jp],
                        func=mybir.ActivationFunctionType.Exp,
                        scale=SCALE,
                    )
                    if jp + 1 < NT // 2:
                        s2s[jp + 1] = emit_scores(jp + 1)
                    elif g + 1 < GL:
                        first_s2 = emit_scores(0, g=g + 1)
                    nc.tensor.matmul(
                        out=u_ps, lhsT=v_sb[:, n0, gsl], rhs=e2[:, 0, :],
                        start=(jp == 0), stop=False,
                    )
                    nc.tensor.matmul(
                        out=u_ps, lhsT=v_sb[:, n1, gsl], rhs=e2[:, 1, :],
                        start=False, stop=(jp == NT // 2 - 1),
                    )
                    # unmasked denominator partials on DVE (2x fp16 mode)
                    if jp == 0:
                        nc.vector.tensor_copy(out=e_aw, in_=e2)
                    else:
                        nc.vector.tensor_add(out=e_aw, in0=e_aw, in1=e2)
                # fold even/odd halves, sum across key partitions, subtract
                # the masked-key count (masked keys contributed exactly 1.0)
                e_fold = rpool.tile([128, 512], FP16, name="e_fold", tag="ef")
                nc.vector.tensor_add(
                    out=e_fold, in0=e_aw[:, 0, :], in1=e_aw[:, 1, :]
                )
                d_tot = dpool.tile([128, 512], FP32, name="d_tot", tag="dt")
                nc.gpsimd.partition_all_reduce(
                    d_tot, e_fold, 128, bass_isa.ReduceOp.add
                )
                nc.vector.tensor_scalar(
                    out=d_tot, in0=d_tot, scalar1=cv_t, scalar2=None,
                    op0=mybir.AluOpType.subtract,
                )
                r_full = dpool.tile([128, 512], FP32, name="r_full", tag="rf")
                with nc.allow_low_precision(reason="softmax reciprocal"):
                    nc.vector.reciprocal(out=r_full, in_=d_tot)
                c_t = cpool.tile([128, 512], FP16, name="c_t", tag="c")
                nc.vector.tensor_mul(out=c_t, in0=u_ps, in1=r_full)
                nc.sync.dma_start(
                    out=ct_loc[sc][g * 128:(g + 1) * 128, :], in_=c_t
                )
            # attention runs act-paced; O work stays out of the PE queue
            # here so chunk completions (which gate the gathers) come as
            # early as possible
            nc.gpsimd.collective_compute(
                "AllGather", mybir.AluOpType.bypass,
                replica_groups=REPL, ins=[ct_loc[sc]], outs=[ct_gth[sc]],
            )
        # ---------- Phase O: all out chunks trail the attention ----------
        for sc in sc_order:
            for tt in range(4):
                emit_o_tt(sc, tt)
        oopool.release()
        ctpool.release()
        cpool.release()
        dpool.release()
        rpool.release()
        awpool.release()
        epool.release()
        ups.release()
        wps.release()
        qld.release()
        qout.release()
        wring.release()
        vres.release()
        kres.release()
        consts.release()

    nc.compile()
    return nc


_NC_CACHE = {}


def _get_program():
    if "nc" not in _NC_CACHE:
        _NC_CACHE["nc"] = _build_program()
    return _NC_CACHE["nc"]


def kernel(query, key, value, mask, position_ids, Wq, Wk, Wv, Wo, **kw):
    query = np.asarray(query, dtype=np.float32)
    key = np.asarray(key, dtype=np.float32)
    value = np.asarray(value, dtype=np.float32)
    mask = np.asarray(mask)
    position_ids = np.asarray(position_ids)
    Wq = np.asarray(Wq, dtype=np.float32)
    Wk = np.asarray(Wk, dtype=np.float32)
    Wv = np.asarray(Wv, dtype=np.float32)
    Wo = np.asarray(Wo, dtype=np.float32)

    # rope tables from actual position_ids (applied to query only)
    pos = position_ids.astype(np.float64)  # (S,)
    freq = np.arange(0, RD, 2, dtype=np.float64)
    inv_freq = 1.0 / (10000.0 ** (freq / RD))  # (RD/2,)
    pe = pos[:, None] * inv_freq[None, :]      # (S, RD/2)
    cos_half = np.cos(pe)
    sin_half = np.sin(pe)
    cosT_full = np.tile(cos_half, (1, 2)).T.astype(np.float16)  # (512, S)
    sinT_signed = np.tile(sin_half, (1, 2)).T.copy()
    sinT_signed[: RD // 2] *= -1.0                              # partner sign
    sinT_signed = sinT_signed.astype(np.float16)
    cosT_id = np.ones((RD, S), np.float16)
    sinT_id = np.zeros((RD, S), np.float16)

    in_maps = []
    for core in range(8):
        b, h = core // 2, core % 2
        hs = slice(h * CL, (h + 1) * CL)
        mv = mask[b].astype(np.float32).reshape(NT, 128).T  # (128, NT)
        n_masked = float((mask[b] == 0).sum())
        in_maps.append({
            "xq": np.ascontiguousarray(query[b].T.astype(np.float16)),
            "xk": np.ascontiguousarray(key[b].T.astype(np.float16)),
            "xv": np.ascontiguousarray(value[b].T.astype(np.float16)),
            "wq": np.ascontiguousarray(Wq[hs, :].T.astype(np.float16)),
            "wk": np.ascontiguousarray(Wk[hs, :].T.astype(np.float16)),
            "wv": np.ascontiguousarray(Wv[hs, :].T.astype(np.float16)),
            "wo": np.ascontiguousarray(Wo.T[:, hs].astype(np.float16)),
            "cosT": cosT_full if h == 0 else cosT_id,
            "sinT": sinT_signed if h == 0 else sinT_id,
            "mvec": np.ascontiguousarray(mv),
            "cvec": np.full((128, 1), n_masked, np.float32),
            "mkey": np.ascontiguousarray(
                np.broadcast_to(mask[b].astype(np.float16), (128, N))
            ),
        })

    nc = _get_program()
    res = run_bass_kernel_spmd(nc, in_maps, core_ids=list(range(8)))
    global LAST_RESULT
    LAST_RESULT = res

    out = np.empty((B, S, D), np.float32)
    for core in range(8):
        b, h = core // 2, core % 2
        out[b][:, h * OC:(h + 1) * OC] = res.results[core]["out"].astype(np.float32)
    return out
